# revision 6
# baseline (speedup 1.0000x reference)
"""DenseNGCN layer on 8 trn2 NeuronCores.

  x = features @ weight                    [50000, 512] @ [512, 64]
  x = A @ x   (twice, A sparse COO E=800k: segment_sum(val * x[col], row))
  out = x + bias

Strategy (dst-node sharding, per the sharding hint):
  - Nodes sharded across 8 cores (6250 rows each, padded to 6272 = 49 blocks
    of 128). Each core owns the projection + SpMM rows of its shard.
  - Edges partitioned by destination row. Per 128-row dst block, edges are
    padded into chunks of 128; per chunk the kernel
      * dma_gathers the 128 source rows x[col] (one 256 B descriptor per
        edge) from the core-local replica of x,
      * loads a host-precomputed one-hot scatter matrix
        S[p, j] = val_p * (rowlocal_p == j) (bf16, dense from HBM),
      * accumulates psum += S.T @ G on the tensor engine,
    which turns the segment-sum into dense matmuls.
  - x is stored as bf16 [N, 64]; rows are gathered in PAIRS (elem = 256 B =
    2 rows) with edges grouped by source parity, so the pair index fits the
    gather's int16 index type and each chunk reads a compile-time half of
    the gathered pair.
  - Full x is re-replicated to every core's HBM between SpMM iterations via
    an AllGather collective (bf16: 6.4 MB).

All edge metadata (sorted/padded pair indices and the dense S tensor) is
precomputed host-side into per-core tensors; the device program is identical
across cores (SPMD).
"""

import numpy as np
import ml_dtypes

N = 50000
E = 800000
IN_CH = 512
OUT_CH = 64
C = 8
P = 128
NSHARD = N // C                # 6250
BLKS = (NSHARD + P - 1) // P   # 49
NPAD = BLKS * P                # 6272
NTOT = NPAD * C                # 50176

_CACHE = {}


def _prep(adj_indices, adj_values):
    """Sort/pad edges into per-(core, dst-block) chunk grids; build gather
    pair-indices and the dense bf16 one-hot S tensor."""
    row = adj_indices[0].astype(np.int64)
    col = adj_indices[1].astype(np.int64)
    val = adj_values.astype(np.float32)

    core = row // NSHARD
    loc = row % NSHARD
    blk = loc // P
    rl = loc % P
    pcol = (col // NSHARD) * NPAD + (col % NSHARD)
    parity = pcol & 1

    key = (core * BLKS + blk) * 2 + parity
    order = np.argsort(key, kind="stable")
    counts = np.bincount(key, minlength=C * BLKS * 2)

    CE = int(-(-counts[0::2].max() // P))
    CO = int(-(-counts[1::2].max() // P))
    KCH = CE + CO
    SLOTS = KCH * P

    start = np.zeros_like(counts)
    start[1:] = np.cumsum(counts)[:-1]
    rank = np.arange(E) - start[key[order]]
    par_s = parity[order]
    slot = rank + par_s * (CE * P)          # slot within the block grid
    c_s = core[order]
    b_s = blk[order]
    ch_s = slot // P                        # chunk within block
    pe_s = slot % P                         # edge partition within chunk

    # gather pair-index grid [C, BLKS, SLOTS]
    g_idx = np.zeros((C, BLKS, SLOTS), np.int64)
    g_idx[c_s, b_s, slot] = pcol[order] >> 1

    # per-(block, parity) max-over-cores edge counts: descriptors beyond a
    # core's count up to max8 are emitted with idx 0 (harmless, val=0); the
    # trailing [max8, group_end) indices are -1 and skipped by the gather.
    cgrid = counts.reshape(C, BLKS, 2)
    max8 = cgrid.max(axis=0)              # [BLKS, 2]
    ar = np.arange(CE * P)
    maskE = (ar[None, None, :] >= cgrid[:, :, 0:1]) & (
        ar[None, None, :] < max8[None, :, 0:1]
    )
    g_idx[:, :, : CE * P][~maskE & (ar[None, None, :] >= cgrid[:, :, 0:1])] = -1
    aro = np.arange(CO * P)
    masko_keep = aro[None, None, :] < max8[None, :, 1:2]
    sl = g_idx[:, :, CE * P :]
    sl[(aro[None, None, :] >= cgrid[:, :, 1:2]) & ~masko_keep] = -1
    g_idx[:, :, CE * P :] = sl

    # wrap: logical i -> [i%16, i//16] per num_idxs=SLOTS gather call
    gi = g_idx.reshape(C, BLKS, KCH * 8, 16)
    gi = gi.transpose(0, 3, 1, 2).reshape(C, 16, BLKS * KCH * 8)
    gall = np.tile(gi, (1, 8, 1)).astype(np.int16)

    # dense one-hot S, partition-major: s[c][pe, (b*KCH+ch)*128 + rl] = val
    s_host = np.zeros((C, P, BLKS * KCH * P), ml_dtypes.bfloat16)
    flat_col = (b_s * KCH + ch_s) * P + rl[order]
    s_host[c_s, pe_s, flat_col] = val[order].astype(ml_dtypes.bfloat16)

    return CE, CO, gall, s_host, max8


def _build(CE, CO, max8):
    import concourse.bacc as bacc
    import concourse.mybir as mybir
    from concourse import tile

    f32 = mybir.dt.float32
    bf16 = mybir.dt.bfloat16
    i16 = mybir.dt.int16
    KCH = CE + CO

    nc = bacc.Bacc(
        None,
        target_bir_lowering=False,
        num_devices=C,
        dynamic_dma_scratch_size=1 << 16,
    )

    featT_d = nc.dram_tensor("featT", [IN_CH, NPAD], bf16, kind="ExternalInput")
    w_d = nc.dram_tensor("w", [IN_CH, OUT_CH], bf16, kind="ExternalInput")
    bias_d = nc.dram_tensor("bias", [P, OUT_CH], f32, kind="ExternalInput")
    gi_d = nc.dram_tensor("gi", [P, BLKS * KCH * 8], i16, kind="ExternalInput")
    s_d = nc.dram_tensor("s", [P, BLKS * KCH * P], bf16, kind="ExternalInput")
    out_d = nc.dram_tensor("out", [NPAD, OUT_CH], f32, kind="ExternalOutput")

    xsh_d = nc.dram_tensor("x_shard", [NPAD, OUT_CH], bf16)
    xA_d = nc.dram_tensor("xA", [NTOT, OUT_CH], bf16)
    xB_d = nc.dram_tensor("xB", [NTOT, OUT_CH], bf16)

    with tile.TileContext(nc) as tc:
        with (
            tc.tile_pool(name="const", bufs=1) as cpool,
            tc.tile_pool(name="g", bufs=3) as gpool,
            tc.tile_pool(name="s", bufs=3) as spool,
            tc.tile_pool(name="o", bufs=3) as opool,
            tc.tile_pool(name="psum", bufs=4, space="PSUM") as pp,
        ):
            w_sb = cpool.tile([P, IN_CH // P, OUT_CH], bf16)
            bias_sb = cpool.tile([P, OUT_CH], f32)
            gi_sb = cpool.tile([P, BLKS * KCH * 8], i16)
            x_sb = cpool.tile([P, BLKS, OUT_CH], bf16)

            nc.sync.dma_start(w_sb[:], w_d[:].rearrange("(k p) c -> p k c", p=P))
            nc.sync.dma_start(bias_sb[:], bias_d[:])
            nc.sync.dma_start(gi_sb[:], gi_d[:])

            for _ in range(3):
                gz = gpool.tile([P, BLKS // BLKS * (CE + CO), 2 * OUT_CH], bf16, tag="G")
                nc.vector.memset(gz[:], 0.0)

            # --- projection: x0 = features @ W for this core's rows ---
            GRP = 7  # blocks per feature-tile group (49 = 7*7)
            with tc.tile_pool(name="feat", bufs=2) as fpool:
                for g in range(BLKS // GRP):
                    feat_sb = fpool.tile([P, IN_CH // P, GRP * P], bf16, tag="f")
                    nc.sync.dma_start(
                        feat_sb[:],
                        featT_d[:, g * GRP * P : (g + 1) * GRP * P].rearrange(
                            "(k p) n -> p k n", p=P
                        ),
                    )
                    for bb in range(GRP):
                        b = g * GRP + bb
                        ps = pp.tile([P, OUT_CH], f32, tag="ps")
                        for k in range(IN_CH // P):
                            nc.tensor.matmul(
                                ps[:],
                                feat_sb[:, k, bb * P : (bb + 1) * P],
                                w_sb[:, k, :],
                                start=(k == 0),
                                stop=(k == IN_CH // P - 1),
                            )
                        nc.vector.tensor_copy(x_sb[:, b, :], ps[:])
                nc.sync.dma_start(
                    xsh_d[:].rearrange("(b p) c -> p b c", p=P), x_sb[:]
                )

            def allgather(dst):
                nc.gpsimd.collective_compute(
                    "AllGather",
                    mybir.AluOpType.bypass,
                    replica_groups=[list(range(C))],
                    ins=[xsh_d[:]],
                    outs=[dst[:]],
                )

            def spmm(src, last):
                # pair view: row i = x[2i] ++ x[2i+1], 256 B
                src_pairs = src[:].rearrange("(a b) c -> a (b c)", b=2)
                for b in range(BLKS):
                    G = gpool.tile([P, KCH, 2 * OUT_CH], bf16, tag="G")
                    nc.gpsimd.dma_gather(
                        G[:, 0:CE, :],
                        src_pairs,
                        gi_sb[:, b * KCH * 8 : b * KCH * 8 + CE * 8],
                        CE * P,
                        int(max8[b, 0]),
                        2 * OUT_CH,
                        single_packet=False,
                    )
                    nc.gpsimd.dma_gather(
                        G[:, CE:KCH, :],
                        src_pairs,
                        gi_sb[:, b * KCH * 8 + CE * 8 : (b + 1) * KCH * 8],
                        CO * P,
                        int(max8[b, 1]),
                        2 * OUT_CH,
                        single_packet=False,
                    )
                    S = spool.tile([P, KCH * P], bf16, tag="S")
                    nc.sync.dma_start(
                        S[:], s_d[:, b * KCH * P : (b + 1) * KCH * P]
                    )
                    ps = pp.tile([P, OUT_CH], f32, tag="ps")
                    for c in range(KCH):
                        off = 0 if c < CE else OUT_CH
                        nc.tensor.matmul(
                            ps[:],
                            S[:, c * P : (c + 1) * P],
                            G[:, c, off : off + OUT_CH],
                            start=(c == 0),
                            stop=(c == KCH - 1),
                        )
                    if last:
                        o = opool.tile([P, OUT_CH], f32, tag="o")
                        nc.vector.tensor_tensor(
                            o[:], ps[:], bias_sb[:], mybir.AluOpType.add
                        )
                        nc.sync.dma_start(out_d[b * P : (b + 1) * P, :], o[:])
                    else:
                        nc.vector.tensor_copy(x_sb[:, b, :], ps[:])
                if not last:
                    nc.sync.dma_start(
                        xsh_d[:].rearrange("(b p) c -> p b c", p=P), x_sb[:]
                    )

            allgather(xA_d)
            spmm(xA_d, last=False)
            allgather(xB_d)
            spmm(xB_d, last=True)

    nc.compile()
    return nc


LAST_RESULT = None


def kernel(adj_indices, adj_values, features, weight, bias):
    global LAST_RESULT
    from concourse.bass_utils import run_bass_kernel_spmd

    CE, CO, gall, s_host, max8 = _prep(
        np.asarray(adj_indices), np.asarray(adj_values)
    )

    key = (CE, CO, max8.tobytes())
    if key not in _CACHE:
        _CACHE[key] = _build(CE, CO, max8)
    nc = _CACHE[key]

    features = np.asarray(features, np.float32)
    weight = np.ascontiguousarray(
        np.asarray(weight, np.float32).astype(ml_dtypes.bfloat16)
    )
    bias128 = np.tile(np.asarray(bias, np.float32).reshape(1, OUT_CH), (P, 1))

    in_maps = []
    for c in range(C):
        featT = np.zeros((IN_CH, NPAD), ml_dtypes.bfloat16)
        featT[:, :NSHARD] = (
            features[c * NSHARD : (c + 1) * NSHARD].T.astype(ml_dtypes.bfloat16)
        )
        in_maps.append(
            {
                "featT": featT,
                "w": weight,
                "bias": bias128,
                "gi": np.ascontiguousarray(gall[c]),
                "s": s_host[c],
            }
        )

    res = run_bass_kernel_spmd(nc, in_maps, core_ids=list(range(C)))
    LAST_RESULT = res

    out = np.concatenate(
        [res.results[c]["out"][:NSHARD] for c in range(C)], axis=0
    )
    return out
